# revision 20
# baseline (speedup 1.0000x reference)
"""FFM CrossLayer pairwise-interaction kernel for 8x Trainium2 NeuronCores.

Math: out[b] = sum_{i<j} <K[i,f_j,:], K[j,f_i,:]> * x[b,i] * x[b,j]
With W[i,j] = sum_o K[i,f_j,o]*K[j,f_i,o] (symmetric), this equals
    out[b] = 0.5 * (x_b^T W x_b - sum_i W[i,i] * x[b,i]^2).

Strategy (8 cores):
  - W build is o-sharded: core c computes the partial W over output-dims
    o in [8c, 8c+8) plus the partial diagonal. The diagonal row is
    AllReduced first (tiny), then each 128-row block of W is AllReduced
    as soon as phase A produces it, overlapping collective transfer with
    the remaining W-build compute.
      term1[j,(i,o)] = K[i,f_j,o] via one-hot matmul on PE (f32r inputs
                       DMA'd directly, no conversion copies)
      term2[j,(i,o)] = K[j,f_i,o] via gpsimd ap_gather (per-partition
                       own-row gather, shared index pattern)
      Z = term1*term2 on DVE, then grouped reduce over o.
  - Main compute is batch-sharded: core c computes y = x_c @ W for its
    512 batch rows (PE), then the whole epilogue
        out = 0.5*sum_j y*x - 0.5*sum_i W_ii*x_i^2
    is ONE fused DVE tensor_tensor_reduce per batch tile (the diag term
    enters as the reduction's initial value from a tiny PE matmul
    against -0.5*W_ii).
"""

import os
import sys

import numpy as np

try:  # the grading env may or may not have concourse on sys.path already
    import concourse.bass as bass  # noqa: F401
except ImportError:
    sys.path.insert(0, "/opt/trn_rl_repo")

import concourse.bacc as bacc
import concourse.bass as bass
import concourse.mybir as mybir
import concourse.tile as tile
from concourse.bass_utils import run_bass_kernel_spmd

B, D, F, O = 4096, 512, 64, 64
NC = 8            # cores
OS = O // NC      # o-slice per core (8)
BS = B // NC      # batch shard per core (512)
P = 128           # partitions
NJT = D // P      # j tiles (4)
F32 = mybir.dt.float32
F32R = mybir.dt.float32r
GROUPS = [list(range(NC))]
SHARED = os.environ.get("KERNEL_SHARED", "0") == "1"
NCHUNK = int(os.environ.get("KERNEL_NCHUNK", "1"))  # W AllReduce chunks
NOCOLL = os.environ.get("KERNEL_NOCOLL", "0") == "1"  # device-health probe
RDIRECT = os.environ.get("KERNEL_RDIRECT", "0") == "1"  # f32r direct-DMA inputs
TTR = os.environ.get("KERNEL_TTR", "1") == "1"  # fused tensor_tensor_reduce

# chunk row ranges over the [D+1, D] logical partial-W matrix
# (row D = diagonal, folded into the last chunk)
_CB = [(ci * (NJT // NCHUNK)) * P for ci in range(NCHUNK)] + [D + 1]
CHUNKS = [(_CB[ci], _CB[ci + 1]) for ci in range(NCHUNK)]

_CACHE = {}


def _build_program(collective=True):
    nc = bacc.Bacc("TRN2", target_bir_lowering=False, debug=False, num_devices=NC)

    FIN = F32R if RDIRECT else F32
    eh = nc.dram_tensor("eh", [F, D], FIN, kind="ExternalInput").ap()
    kt = nc.dram_tensor("kt", [F, D * OS], FIN, kind="ExternalInput").ap()
    idx = nc.dram_tensor("idx", [P, D // 16], mybir.dt.int16, kind="ExternalInput").ap()
    ko = nc.dram_tensor("ko", [D, F * OS], F32, kind="ExternalInput").ap()
    eif = nc.dram_tensor("eif", [D, F], F32, kind="ExternalInput").ap()
    xT = nc.dram_tensor("xT", [D, BS], FIN, kind="ExternalInput").ap()
    xc = nc.dram_tensor("xc", [BS, D], F32, kind="ExternalInput").ap()
    outv = nc.dram_tensor("outv", [BS], F32, kind="ExternalOutput").ap()

    out_space = "Shared" if (collective and SHARED) else "Local"

    with tile.TileContext(nc) as tc:
        with (
            tc.tile_pool(name="cst", bufs=1) as cst,
            tc.tile_pool(name="sb", bufs=2) as sb,
            tc.tile_pool(name="wpool", bufs=1) as wpool,
            tc.tile_pool(name="psA", bufs=2, space="PSUM") as psA,
            tc.tile_pool(name="psY", bufs=2, space="PSUM") as psY,
            tc.tile_pool(name="psD", bufs=2, space="PSUM") as psD,
            tc.tile_pool(name="dram", bufs=1, space="DRAM") as dram,
        ):
            # ---- constant loads (phase A inputs first, phase B after) ----
            if RDIRECT:
                eh_sb = cst.tile([F, D], F32R, name="eh_sb")
                nc.sync.dma_start(eh_sb[:], eh[:])
                kt_sb = cst.tile([F, D * OS], F32R, name="kt_sb")
                nc.sync.dma_start(kt_sb[:], kt[:])
            else:
                eh_f = cst.tile([F, D], F32)
                nc.sync.dma_start(eh_f[:], eh[:])
                eh_sb = cst.tile([F, D], F32R, name="eh_sb")
                nc.scalar.copy(eh_sb[:], eh_f[:])
                kt_f = cst.tile([F, D * OS], F32)
                nc.sync.dma_start(kt_f[:], kt[:])
                kt_sb = cst.tile([F, D * OS], F32R, name="kt_sb")
                nc.scalar.copy(kt_sb[:], kt_f[:])
            idx_sb = cst.tile([P, D // 16], mybir.dt.int16)
            nc.sync.dma_start(idx_sb[:], idx[:])
            ko_sb = []
            for jt in range(NJT):
                t = cst.tile([P, F * OS], F32, tag=f"ko{jt}")
                nc.sync.dma_start(t[:], ko[jt * P : (jt + 1) * P, :])
                ko_sb.append(t)
            eif_sb = []
            for it in range(NJT):
                t = cst.tile([P, F], F32, tag=f"eif{it}")
                nc.sync.dma_start(t[:], eif[it * P : (it + 1) * P, :])
                eif_sb.append(t)
            xT_sb = []
            xTf_sb = []
            for jc in range(NJT):
                if RDIRECT:
                    t = cst.tile([P, BS], F32R, tag=f"xT{jc}", name=f"xT{jc}")
                    nc.sync.dma_start(t[:], xT[jc * P : (jc + 1) * P, :])
                    xTf_sb.append(t)
                else:
                    tf = cst.tile([P, BS], F32, tag=f"xTf{jc}", name=f"xTf{jc}")
                    nc.sync.dma_start(tf[:], xT[jc * P : (jc + 1) * P, :])
                    t = cst.tile([P, BS], F32R, tag=f"xT{jc}", name=f"xT{jc}")
                    nc.scalar.copy(t[:], tf[:])
                    xTf_sb.append(tf)
                xT_sb.append(t)
            xc_sb = []
            for bt in range(NJT):
                t = cst.tile([P, D], F32, tag=f"xc{bt}")
                nc.sync.dma_start(t[:], xc[bt * P : (bt + 1) * P, :])
                xc_sb.append(t)

            # per-chunk DRAM staging for the AllReduces; the diagonal is
            # row D of the logical [D+1, D] partial matrix (last chunk).
            wpd = [
                dram.tile([r1 - r0, D], F32, tag=f"wpd{ci}", name=f"wpd{ci}")
                for ci, (r0, r1) in enumerate(CHUNKS)
            ]
            wrd = [
                dram.tile([r1 - r0, D], F32, tag=f"wrd{ci}", name=f"wrd{ci}",
                          addr_space=out_space)
                for ci, (r0, r1) in enumerate(CHUNKS)
            ]

            def chunk_of(row):  # -> (chunk idx, local row)
                for ci, (r0, r1) in enumerate(CHUNKS):
                    if r0 <= row < r1:
                        return ci, row - r0
                raise AssertionError(row)

            # ---- diagonal first: W_ii = sum_{o in slice} K[i, f_i, o]^2 ----
            dci, dloc = chunk_of(D)
            for it in range(NJT):
                sq = sb.tile([P, F * OS], F32, tag="sq")
                nc.scalar.square(sq[:], ko_sb[it][:])
                sqr = sb.tile([P, F], F32, tag="sqr")
                nc.vector.tensor_reduce(
                    sqr[:], sq[:].rearrange("p (f o) -> p f o", o=OS),
                    axis=mybir.AxisListType.X, op=mybir.AluOpType.add,
                )
                junkd = sb.tile([P, F], F32, tag="junkd")
                dcol = sb.tile([P, 1], F32, tag="dcol")
                if TTR:
                    nc.vector.tensor_tensor_reduce(
                        junkd[:], sqr[:], eif_sb[it][:], 1.0, 0.0,
                        mybir.AluOpType.mult, mybir.AluOpType.add, dcol[:],
                    )
                else:
                    nc.vector.tensor_mul(junkd[:], sqr[:], eif_sb[it][:])
                    nc.vector.tensor_reduce(
                        dcol[:], junkd[:],
                        axis=mybir.AxisListType.X, op=mybir.AluOpType.add,
                    )
                nc.sync.dma_start(
                    wpd[dci][dloc : dloc + 1, it * P : (it + 1) * P], dcol[:]
                )

            # ---- phase A: partial W (o-slice), chunked AllReduce ----
            for jt in range(NJT):
                t2 = sb.tile([P, D * OS], F32, tag="t2")
                nc.gpsimd.ap_gather(
                    t2[:], ko_sb[jt][:], idx_sb[:],
                    channels=P, num_elems=F, d=OS, num_idxs=D,
                )
                w_t = sb.tile([P, D], F32, tag="wt")
                for q in range(4):  # quarters of the (i,o) axis: 128 i each
                    pt = psA.tile([P, P * OS], F32, tag="pt")  # [128,1024]
                    for n in range(2):
                        nc.tensor.matmul(
                            pt[:, n * 512 : (n + 1) * 512],
                            eh_sb[:, jt * P : (jt + 1) * P],
                            kt_sb[:, q * P * OS + n * 512 : q * P * OS + (n + 1) * 512],
                            start=True, stop=True,
                        )
                    z = sb.tile([P, P * OS], F32, tag="z")
                    nc.vector.tensor_mul(z[:], pt[:], t2[:, q * P * OS : (q + 1) * P * OS])
                    zv = z[:].rearrange("p (i o) -> p i o", o=OS)
                    nc.vector.tensor_reduce(
                        w_t[:, q * P : (q + 1) * P], zv,
                        axis=mybir.AxisListType.X, op=mybir.AluOpType.add,
                    )
                bci, bloc = chunk_of(jt * P)
                nc.sync.dma_start(wpd[bci][bloc : bloc + P, :], w_t[:])
                # fire the chunk's AllReduce once its last row block is in
                if (jt + 1) * P == CHUNKS[bci][1] or (
                    bci == dci and (jt + 1) * P == CHUNKS[bci][1] - 1
                ):
                    if collective:
                        nc.gpsimd.collective_compute(
                            "AllReduce", mybir.AluOpType.add,
                            replica_groups=GROUPS,
                            ins=[wpd[bci][:]], outs=[wrd[bci][:]],
                        )
                    else:
                        nc.sync.dma_start(wrd[bci][:], wpd[bci][:])

            # ---- phase B inputs: reduced W blocks (f32r), -0.5*diag, x^2 ----
            w_sb = []
            dcol_sb = []
            for jc in range(NJT):
                bci, bloc = chunk_of(jc * P)
                wf = sb.tile([P, D], F32, tag="wf")
                nc.sync.dma_start(wf[:], wrd[bci][bloc : bloc + P, :])
                wr = wpool.tile([P, D], F32R, tag=f"w{jc}")
                nc.scalar.copy(wr[:], wf[:])
                w_sb.append(wr)
                dn = sb.tile([P, 1], F32, tag="dn")
                nc.sync.dma_start(
                    dn[:], wrd[dci][dloc : dloc + 1, jc * P : (jc + 1) * P]
                )
                dneg = wpool.tile([P, 1], F32, tag=f"d{jc}")
                nc.scalar.mul(dneg[:], dn[:], -0.5)
                dcol_sb.append(dneg)
            xsq_sb = []
            for jc in range(NJT):
                t = cst.tile([P, BS], F32, tag=f"xsq{jc}", name=f"xsq{jc}")
                nc.scalar.square(t[:], xTf_sb[jc][:])
                xsq_sb.append(t)

            # ---- phase B: y = x_c @ W, fused epilogue ----
            for bt in range(NJT):
                yp = psY.tile([P, D], F32, tag="yp")
                for jc in range(NJT):
                    nc.tensor.matmul(
                        yp[:], xT_sb[jc][:, bt * P : (bt + 1) * P], w_sb[jc][:],
                        start=(jc == 0), stop=(jc == NJT - 1),
                    )
                y2p = psD.tile([P, 1], F32, tag="y2p")
                for it in range(NJT):
                    nc.tensor.matmul(
                        y2p[:], xsq_sb[it][:, bt * P : (bt + 1) * P], dcol_sb[it][:],
                        start=(it == 0), stop=(it == NJT - 1),
                    )
                junk2 = sb.tile([P, D], F32, tag="junk2")
                ov = sb.tile([P, 1], F32, tag="ov")
                if TTR:
                    nc.vector.tensor_tensor_reduce(
                        junk2[:], yp[:], xc_sb[bt][:], 0.5, y2p[:],
                        mybir.AluOpType.mult, mybir.AluOpType.add, ov[:],
                    )
                else:
                    nc.vector.tensor_mul(junk2[:], yp[:], xc_sb[bt][:])
                    sres = sb.tile([P, 1], F32, tag="sres")
                    nc.vector.tensor_reduce(
                        sres[:], junk2[:],
                        axis=mybir.AxisListType.X, op=mybir.AluOpType.add,
                    )
                    hres = sb.tile([P, 1], F32, tag="hres")
                    nc.scalar.mul(hres[:], sres[:], 0.5)
                    nc.vector.tensor_add(ov[:], hres[:], y2p[:])
                nc.sync.dma_start(outv[bt * P : (bt + 1) * P], ov[:])

    nc.compile()
    return nc


def _host_prep(x, kern, field_ids):
    x = np.ascontiguousarray(np.asarray(x, dtype=np.float32))
    k = np.ascontiguousarray(np.asarray(kern, dtype=np.float32))
    fid = np.asarray(field_ids).astype(np.int64).ravel()
    assert x.shape == (B, D) and k.shape == (D, F, O) and fid.shape == (D,)

    ehot_fj = (fid[None, :] == np.arange(F)[:, None]).astype(np.float32)  # [F, D]
    ehot_if = np.ascontiguousarray(ehot_fj.T)                              # [D, F]
    idx16 = np.zeros((16, D // 16), np.int16)
    for kk in range(D):
        idx16[kk % 16, kk // 16] = fid[kk]
    idx_w = np.tile(idx16, (P // 16, 1))

    in_maps = []
    for c in range(NC):
        ksl = k[:, :, c * OS : (c + 1) * OS]                   # [D, F, OS]
        kt_c = np.ascontiguousarray(ksl.transpose(1, 0, 2)).reshape(F, D * OS)
        ko_c = np.ascontiguousarray(ksl).reshape(D, F * OS)
        xs = x[c * BS : (c + 1) * BS]
        in_maps.append({
            "eh": ehot_fj, "eif": ehot_if,
            "kt": kt_c, "ko": ko_c, "idx": idx_w,
            "xT": np.ascontiguousarray(xs.T), "xc": xs,
        })
    return in_maps


def kernel(x, kernel, field_ids):
    key = ("nc", NCHUNK, SHARED, NOCOLL, RDIRECT, TTR)
    if key not in _CACHE:
        _CACHE[key] = _build_program(collective=not NOCOLL)
    nc = _CACHE[key]
    in_maps = _host_prep(x, kernel, field_ids)
    kw = {}
    if os.environ.get("KERNEL_TRACE") == "1":
        kw["trace"] = True
        td = os.environ.get("KERNEL_TRACE_DIR")
        if td:
            os.makedirs(td, exist_ok=True)
            kw["tmpdir"] = td
        tc_env = os.environ.get("KERNEL_TRACE_CORES")
        if tc_env:
            kw["trace_cores"] = [int(c) for c in tc_env.split(",")]
    res = run_bass_kernel_spmd(nc, in_maps, core_ids=list(range(NC)), **kw)
    _CACHE["last_res"] = res
    if res.exec_time_ns is not None:
        _CACHE["hw_ns"] = res.exec_time_ns
    out = np.concatenate([np.asarray(res.results[c]["outv"]).ravel() for c in range(NC)])
    return out.astype(np.float32)


# revision 23
# speedup vs baseline: 1.1227x; 1.1227x over previous
"""FFM CrossLayer pairwise-interaction kernel for 8x Trainium2 NeuronCores.

Math: out[b] = sum_{i<j} <K[i,f_j,:], K[j,f_i,:]> * x[b,i] * x[b,j]
With W[i,j] = sum_o K[i,f_j,o]*K[j,f_i,o] (symmetric), this equals
    out[b] = 0.5 * (x_b^T W x_b - sum_i W[i,i] * x[b,i]^2).

Strategy (8 cores):
  - W build is o-sharded: core c computes the partial W over output-dims
    o in [8c, 8c+8) plus the partial diagonal. The diagonal row is
    AllReduced first (tiny), then each 128-row block of W is AllReduced
    as soon as phase A produces it, overlapping collective transfer with
    the remaining W-build compute.
      term1[j,(i,o)] = K[i,f_j,o] via one-hot matmul on PE (f32r inputs
                       DMA'd directly, no conversion copies)
      term2[j,(i,o)] = K[j,f_i,o] via gpsimd ap_gather (per-partition
                       own-row gather, shared index pattern)
      Z = term1*term2 on DVE, then grouped reduce over o.
  - Main compute is batch-sharded: core c computes y = x_c @ W for its
    512 batch rows (PE), then the whole epilogue
        out = 0.5*sum_j y*x - 0.5*sum_i W_ii*x_i^2
    is ONE fused DVE tensor_tensor_reduce per batch tile (the diag term
    enters as the reduction's initial value from a tiny PE matmul
    against -0.5*W_ii).
"""

import os
import sys

import numpy as np

try:  # the grading env may or may not have concourse on sys.path already
    import concourse.bass as bass  # noqa: F401
except ImportError:
    sys.path.insert(0, "/opt/trn_rl_repo")

import concourse.bacc as bacc
import concourse.bass as bass
import concourse.mybir as mybir
import concourse.tile as tile
from concourse.bass_utils import run_bass_kernel_spmd

B, D, F, O = 4096, 512, 64, 64
NC = 8            # cores
OS = O // NC      # o-slice per core (8)
BS = B // NC      # batch shard per core (512)
P = 128           # partitions
NJT = D // P      # j tiles (4)
F32 = mybir.dt.float32
F32R = mybir.dt.float32r
GROUPS = [list(range(NC))]
SHARED = os.environ.get("KERNEL_SHARED", "0") == "1"
NCHUNK = int(os.environ.get("KERNEL_NCHUNK", "1"))  # W AllReduce chunks
NOCOLL = os.environ.get("KERNEL_NOCOLL", "0") == "1"  # device-health probe
RDIRECT = os.environ.get("KERNEL_RDIRECT", "0") == "1"  # f32r direct-DMA inputs
TTR = os.environ.get("KERNEL_TTR", "1") == "1"  # fused tensor_tensor_reduce

# chunk row ranges over the [D+1, D] logical partial-W matrix
# (row D = diagonal, folded into the last chunk)
_CB = [(ci * (NJT // NCHUNK)) * P for ci in range(NCHUNK)] + [D + 1]
CHUNKS = [(_CB[ci], _CB[ci + 1]) for ci in range(NCHUNK)]

_CACHE = {}


def _build_program(collective=True):
    nc = bacc.Bacc("TRN2", target_bir_lowering=False, debug=False, num_devices=NC)

    FIN = F32R if RDIRECT else F32
    eh = nc.dram_tensor("eh", [F, D], FIN, kind="ExternalInput").ap()
    kt = nc.dram_tensor("kt", [F, D * OS], FIN, kind="ExternalInput").ap()
    idx = nc.dram_tensor("idx", [P, D // 16], mybir.dt.int16, kind="ExternalInput").ap()
    ko = nc.dram_tensor("ko", [D, F * OS], F32, kind="ExternalInput").ap()
    eif = nc.dram_tensor("eif", [D, F], F32, kind="ExternalInput").ap()
    xT = nc.dram_tensor("xT", [D, BS], FIN, kind="ExternalInput").ap()
    xc = nc.dram_tensor("xc", [BS, D], F32, kind="ExternalInput").ap()
    outv = nc.dram_tensor("outv", [BS], F32, kind="ExternalOutput").ap()

    out_space = "Shared" if (collective and SHARED) else "Local"

    with tile.TileContext(nc) as tc:
        with (
            tc.tile_pool(name="cst", bufs=1) as cst,
            tc.tile_pool(name="sb", bufs=2) as sb,
            tc.tile_pool(name="wpool", bufs=1) as wpool,
            tc.tile_pool(name="psA", bufs=2, space="PSUM") as psA,
            tc.tile_pool(name="psY", bufs=2, space="PSUM") as psY,
            tc.tile_pool(name="psD", bufs=2, space="PSUM") as psD,
            tc.tile_pool(name="dram", bufs=1, space="DRAM") as dram,
        ):
            # ---- constant loads ----
            # phase-A-critical loads on the sync (SP) DMA queue, in the
            # order phase A consumes them: idx+ko gate the first gather,
            # eh+kt gate the first matmul. Phase-B loads go on the scalar
            # (Activation) queue so they don't delay phase A.
            idx_sb = cst.tile([P, D // 16], mybir.dt.int16)
            nc.sync.dma_start(idx_sb[:], idx[:])
            ko_sb = []
            for jt in range(NJT):
                t = cst.tile([P, F * OS], F32, tag=f"ko{jt}")
                nc.sync.dma_start(t[:], ko[jt * P : (jt + 1) * P, :])
                ko_sb.append(t)
            if RDIRECT:
                eh_sb = cst.tile([F, D], F32R, name="eh_sb")
                nc.sync.dma_start(eh_sb[:], eh[:])
                kt_sb = cst.tile([F, D * OS], F32R, name="kt_sb")
                nc.sync.dma_start(kt_sb[:], kt[:])
            else:
                eh_f = cst.tile([F, D], F32)
                nc.sync.dma_start(eh_f[:], eh[:])
                eh_sb = cst.tile([F, D], F32R, name="eh_sb")
                nc.scalar.copy(eh_sb[:], eh_f[:])
                kt_f = cst.tile([F, D * OS], F32)
                nc.sync.dma_start(kt_f[:], kt[:])
                kt_sb = cst.tile([F, D * OS], F32R, name="kt_sb")
                nc.scalar.copy(kt_sb[:], kt_f[:])
            eif_sb = []
            for it in range(NJT):
                t = cst.tile([P, F], F32, tag=f"eif{it}")
                nc.scalar.dma_start(t[:], eif[it * P : (it + 1) * P, :])
                eif_sb.append(t)
            xT_sb = []
            xTf_sb = []
            for jc in range(NJT):
                if RDIRECT:
                    t = cst.tile([P, BS], F32R, tag=f"xT{jc}", name=f"xT{jc}")
                    nc.scalar.dma_start(t[:], xT[jc * P : (jc + 1) * P, :])
                    xTf_sb.append(t)
                else:
                    tf = cst.tile([P, BS], F32, tag=f"xTf{jc}", name=f"xTf{jc}")
                    nc.scalar.dma_start(tf[:], xT[jc * P : (jc + 1) * P, :])
                    t = cst.tile([P, BS], F32R, tag=f"xT{jc}", name=f"xT{jc}")
                    nc.scalar.copy(t[:], tf[:])
                    xTf_sb.append(tf)
                xT_sb.append(t)
            xc_sb = []
            for bt in range(NJT):
                t = cst.tile([P, D], F32, tag=f"xc{bt}")
                nc.scalar.dma_start(t[:], xc[bt * P : (bt + 1) * P, :])
                xc_sb.append(t)

            # per-chunk DRAM staging for the AllReduces; the diagonal is
            # row D of the logical [D+1, D] partial matrix (last chunk).
            wpd = [
                dram.tile([r1 - r0, D], F32, tag=f"wpd{ci}", name=f"wpd{ci}")
                for ci, (r0, r1) in enumerate(CHUNKS)
            ]
            wrd = [
                dram.tile([r1 - r0, D], F32, tag=f"wrd{ci}", name=f"wrd{ci}",
                          addr_space=out_space)
                for ci, (r0, r1) in enumerate(CHUNKS)
            ]

            def chunk_of(row):  # -> (chunk idx, local row)
                for ci, (r0, r1) in enumerate(CHUNKS):
                    if r0 <= row < r1:
                        return ci, row - r0
                raise AssertionError(row)

            # ---- diagonal first: W_ii = sum_{o in slice} K[i, f_i, o]^2 ----
            dci, dloc = chunk_of(D)
            for it in range(NJT):
                sq = sb.tile([P, F * OS], F32, tag="sq")
                nc.scalar.square(sq[:], ko_sb[it][:])
                sqr = sb.tile([P, F], F32, tag="sqr")
                nc.vector.tensor_reduce(
                    sqr[:], sq[:].rearrange("p (f o) -> p f o", o=OS),
                    axis=mybir.AxisListType.X, op=mybir.AluOpType.add,
                )
                junkd = sb.tile([P, F], F32, tag="junkd")
                dcol = sb.tile([P, 1], F32, tag="dcol")
                if TTR:
                    nc.vector.tensor_tensor_reduce(
                        junkd[:], sqr[:], eif_sb[it][:], 1.0, 0.0,
                        mybir.AluOpType.mult, mybir.AluOpType.add, dcol[:],
                    )
                else:
                    nc.vector.tensor_mul(junkd[:], sqr[:], eif_sb[it][:])
                    nc.vector.tensor_reduce(
                        dcol[:], junkd[:],
                        axis=mybir.AxisListType.X, op=mybir.AluOpType.add,
                    )
                nc.sync.dma_start(
                    wpd[dci][dloc : dloc + 1, it * P : (it + 1) * P], dcol[:]
                )

            # ---- phase A: partial W (o-slice), chunked AllReduce ----
            for jt in range(NJT):
                t2 = sb.tile([P, D * OS], F32, tag="t2", bufs=3)
                nc.gpsimd.ap_gather(
                    t2[:], ko_sb[jt][:], idx_sb[:],
                    channels=P, num_elems=F, d=OS, num_idxs=D,
                )
                w_t = sb.tile([P, D], F32, tag="wt", bufs=3)
                for q in range(4):  # quarters of the (i,o) axis: 128 i each
                    pt = psA.tile([P, P * OS], F32, tag="pt")  # [128,1024]
                    for n in range(2):
                        nc.tensor.matmul(
                            pt[:, n * 512 : (n + 1) * 512],
                            eh_sb[:, jt * P : (jt + 1) * P],
                            kt_sb[:, q * P * OS + n * 512 : q * P * OS + (n + 1) * 512],
                            start=True, stop=True,
                        )
                    z = sb.tile([P, P * OS], F32, tag="z", bufs=3)
                    nc.vector.tensor_mul(z[:], pt[:], t2[:, q * P * OS : (q + 1) * P * OS])
                    zv = z[:].rearrange("p (i o) -> p i o", o=OS)
                    nc.vector.tensor_reduce(
                        w_t[:, q * P : (q + 1) * P], zv,
                        axis=mybir.AxisListType.X, op=mybir.AluOpType.add,
                    )
                bci, bloc = chunk_of(jt * P)
                nc.sync.dma_start(wpd[bci][bloc : bloc + P, :], w_t[:])
                # fire the chunk's AllReduce once its last row block is in
                if (jt + 1) * P == CHUNKS[bci][1] or (
                    bci == dci and (jt + 1) * P == CHUNKS[bci][1] - 1
                ):
                    if collective:
                        nc.gpsimd.collective_compute(
                            "AllReduce", mybir.AluOpType.add,
                            replica_groups=GROUPS,
                            ins=[wpd[bci][:]], outs=[wrd[bci][:]],
                        )
                    else:
                        nc.sync.dma_start(wrd[bci][:], wpd[bci][:])

            # ---- phase B inputs: reduced W blocks (f32r), -0.5*diag, x^2 ----
            w_sb = []
            dcol_sb = []
            for jc in range(NJT):
                bci, bloc = chunk_of(jc * P)
                wf = sb.tile([P, D], F32, tag="wf", bufs=4)
                nc.scalar.dma_start(wf[:], wrd[bci][bloc : bloc + P, :])
                wr = wpool.tile([P, D], F32R, tag=f"w{jc}")
                nc.scalar.copy(wr[:], wf[:])
                w_sb.append(wr)
                dn = sb.tile([P, 1], F32, tag="dn", bufs=4)
                nc.scalar.dma_start(
                    dn[:], wrd[dci][dloc : dloc + 1, jc * P : (jc + 1) * P]
                )
                dneg = wpool.tile([P, 1], F32, tag=f"d{jc}")
                nc.scalar.mul(dneg[:], dn[:], -0.5)
                dcol_sb.append(dneg)
            xsq_sb = []
            for jc in range(NJT):
                t = cst.tile([P, BS], F32, tag=f"xsq{jc}", name=f"xsq{jc}")
                nc.scalar.square(t[:], xTf_sb[jc][:])
                xsq_sb.append(t)

            # ---- phase B: y = x_c @ W, fused epilogue ----
            for bt in range(NJT):
                yp = psY.tile([P, D], F32, tag="yp")
                for jc in range(NJT):
                    nc.tensor.matmul(
                        yp[:], xT_sb[jc][:, bt * P : (bt + 1) * P], w_sb[jc][:],
                        start=(jc == 0), stop=(jc == NJT - 1),
                    )
                y2p = psD.tile([P, 1], F32, tag="y2p")
                for it in range(NJT):
                    nc.tensor.matmul(
                        y2p[:], xsq_sb[it][:, bt * P : (bt + 1) * P], dcol_sb[it][:],
                        start=(it == 0), stop=(it == NJT - 1),
                    )
                junk2 = sb.tile([P, D], F32, tag="junk2")
                ov = sb.tile([P, 1], F32, tag="ov")
                if TTR:
                    nc.vector.tensor_tensor_reduce(
                        junk2[:], yp[:], xc_sb[bt][:], 0.5, y2p[:],
                        mybir.AluOpType.mult, mybir.AluOpType.add, ov[:],
                    )
                else:
                    nc.vector.tensor_mul(junk2[:], yp[:], xc_sb[bt][:])
                    sres = sb.tile([P, 1], F32, tag="sres")
                    nc.vector.tensor_reduce(
                        sres[:], junk2[:],
                        axis=mybir.AxisListType.X, op=mybir.AluOpType.add,
                    )
                    hres = sb.tile([P, 1], F32, tag="hres")
                    nc.scalar.mul(hres[:], sres[:], 0.5)
                    nc.vector.tensor_add(ov[:], hres[:], y2p[:])
                nc.sync.dma_start(outv[bt * P : (bt + 1) * P], ov[:])

    nc.compile()
    return nc


def _host_prep(x, kern, field_ids):
    x = np.ascontiguousarray(np.asarray(x, dtype=np.float32))
    k = np.ascontiguousarray(np.asarray(kern, dtype=np.float32))
    fid = np.asarray(field_ids).astype(np.int64).ravel()
    assert x.shape == (B, D) and k.shape == (D, F, O) and fid.shape == (D,)

    ehot_fj = (fid[None, :] == np.arange(F)[:, None]).astype(np.float32)  # [F, D]
    ehot_if = np.ascontiguousarray(ehot_fj.T)                              # [D, F]
    idx16 = np.zeros((16, D // 16), np.int16)
    for kk in range(D):
        idx16[kk % 16, kk // 16] = fid[kk]
    idx_w = np.tile(idx16, (P // 16, 1))

    in_maps = []
    for c in range(NC):
        ksl = k[:, :, c * OS : (c + 1) * OS]                   # [D, F, OS]
        kt_c = np.ascontiguousarray(ksl.transpose(1, 0, 2)).reshape(F, D * OS)
        ko_c = np.ascontiguousarray(ksl).reshape(D, F * OS)
        xs = x[c * BS : (c + 1) * BS]
        in_maps.append({
            "eh": ehot_fj, "eif": ehot_if,
            "kt": kt_c, "ko": ko_c, "idx": idx_w,
            "xT": np.ascontiguousarray(xs.T), "xc": xs,
        })
    return in_maps


def kernel(x, kernel, field_ids):
    key = ("nc", NCHUNK, SHARED, NOCOLL, RDIRECT, TTR)
    if key not in _CACHE:
        _CACHE[key] = _build_program(collective=not NOCOLL)
    nc = _CACHE[key]
    in_maps = _host_prep(x, kernel, field_ids)
    kw = {}
    if os.environ.get("KERNEL_TRACE") == "1":
        kw["trace"] = True
        td = os.environ.get("KERNEL_TRACE_DIR")
        if td:
            os.makedirs(td, exist_ok=True)
            kw["tmpdir"] = td
        tc_env = os.environ.get("KERNEL_TRACE_CORES")
        if tc_env:
            kw["trace_cores"] = [int(c) for c in tc_env.split(",")]
    res = run_bass_kernel_spmd(nc, in_maps, core_ids=list(range(NC)), **kw)
    _CACHE["last_res"] = res
    if res.exec_time_ns is not None:
        _CACHE["hw_ns"] = res.exec_time_ns
    out = np.concatenate([np.asarray(res.results[c]["outv"]).ravel() for c in range(NC)])
    return out.astype(np.float32)
